# revision 3
# baseline (speedup 1.0000x reference)
"""Harmonic-comb attention kernel for 8 Trainium2 NeuronCores — v2.

Data-parallel over (batch, time-half): core i handles b = i // 2,
t in [256*(i%2), ...+256).  All convs are 1x3 along f, so t shards with
no halo.  Weights and the comb matrix are replicated per core.

Key optimizations over v1 (all validated in numpy vs the f32 reference):
 - The 360x100 comb matrix Q has only ~100 DISTINCT rows (multiplicity up
   to 20).  Softmax over duplicated rows collapses exactly:
   h = sum_d m_d exp(s_d) Q_d / sum_d m_d exp(s_d), with the multiplicity
   folded into the exp as a per-partition bias log(m_d).  This cuts the
   scores matmul, the exp, and the h-projection by 3.6x.
 - conv_k bias bk folds into the scores matmul as a rank-1 term: an extra
   constant row bk[mc] in koT (K=101) against a row of Q-row-sums in the
   lhsT.  k_out needs no bias -> its PSUM->SBUF move is a pure cast.
 - conv_k is computed activations-stationary so k_out comes out of the PE
   transposed [f, mc] — exactly the scores-rhs layout; the per-position
   xbar-transpose DMAs of v1 are gone.
 - Elementwise ops run on a 2-sub stacked layout [128 = 2x64 channels]
   halving their column counts; work is spread across ACT/DVE only — the
   GpSimd/Pool engine is ~10x slower than its cost model on real HW and is
   used for nothing but one-time init memsets (measured: -180us).
 - Normalization is batched: one strided reciprocal per sub and one
   stride-0-broadcast multiply, instead of per-position ops.
 - Conv biases ride per-partition in the PSUM->SBUF drains (no ones rows).
 - The macro loop is software-pipelined over 7 stages (skewed issue) so
   each in-order engine interleaves work from different macros (2.2x).
 - x / y ride in host-pre-tiled DRAM layouts so every DMA is one
   contiguous region (DGE spray) instead of 64-128 strided descriptor
   runs (measured: -80us).
"""

import contextlib
import sys

sys.path.insert(0, "/opt/trn_rl_repo")

import numpy as np
import ml_dtypes

import concourse.bacc as bacc
import concourse.mybir as mybir
import concourse.tile as tile
from concourse.bass import AP
from concourse.bass_utils import run_bass_kernel_spmd

BF16 = ml_dtypes.bfloat16
F32 = mybir.dt.float32
BF = mybir.dt.bfloat16
AF = mybir.ActivationFunctionType
ALU = mybir.AluOpType

C = 64        # input channels
MC = 128      # attention channels
F = 100       # freq bins
FB = 102      # padded block stride (zero | 100 data | zero)
MACRO = 8     # positions per macro batch (2 stacked subs of 4)
SUB = 4       # positions per sub
ND = 100      # distinct comb rows (asserted at runtime)

_cache = {}


def _build(t_core, repeat=1):
    assert t_core % MACRO == 0
    nmacro = t_core // MACRO

    nc = bacc.Bacc("TRN2", target_bir_lowering=False, debug=False)

    x_d = nc.dram_tensor("x", [nmacro * 128, SUB * F], F32,
                         kind="ExternalInput").ap()
    x2_d = nc.dram_tensor("x2", [nmacro * C, MACRO * F], F32,
                          kind="ExternalInput").ap()
    wv_d = nc.dram_tensor("wv", [128, 3 * MC], BF, kind="ExternalInput").ap()
    wk_d = nc.dram_tensor("wk", [128, 3 * MC], BF, kind="ExternalInput").ap()
    wkq_d = nc.dram_tensor("wkq", [MC, 3 * MC], BF, kind="ExternalInput").ap()
    wo_d = nc.dram_tensor("wo", [MC, 3 * C], BF, kind="ExternalInput").ap()
    hmt_d = nc.dram_tensor("hmt", [ND + 1, ND], BF, kind="ExternalInput").ap()
    hma_d = nc.dram_tensor("hma", [ND, ND + 1], BF, kind="ExternalInput").ap()
    logm_d = nc.dram_tensor("logm", [ND, 1], F32, kind="ExternalInput").ap()
    bkrow_d = nc.dram_tensor("bkrow", [1, MACRO * MC], BF,
                             kind="ExternalInput").ap()
    nav_d = nc.dram_tensor("nav", [128, 1], F32, kind="ExternalInput").ap()
    nbv_d = nc.dram_tensor("nbv", [128, 1], F32, kind="ExternalInput").ap()
    t2s_d = nc.dram_tensor("t2s", [128, 1], F32, kind="ExternalInput").ap()
    t2b_d = nc.dram_tensor("t2b", [128, 1], F32, kind="ExternalInput").ap()
    bv_d = nc.dram_tensor("bvv", [MC, 1], F32, kind="ExternalInput").ap()
    bkq_d = nc.dram_tensor("bkqv", [MC, 1], F32, kind="ExternalInput").ap()
    bo_d = nc.dram_tensor("bov", [C, 1], F32, kind="ExternalInput").ap()
    y_d = nc.dram_tensor("y", [nmacro * C, MACRO * F], F32,
                         kind="ExternalOutput").ap()

    # persistent padded tiles (pads zeroed once; data rewritten per macro)
    t_pad = [nc.alloc_sbuf_tensor(f"t_pad{i}", [128, SUB * FB], BF).ap()
             for i in range(3)]
    kx_pad = [nc.alloc_sbuf_tensor(f"kx_pad{i}", [128, SUB * FB], BF).ap()
              for i in range(3)]
    hq_pad = [nc.alloc_sbuf_tensor(f"hq_pad{i}", [MC, MACRO * FB], BF).ap()
              for i in range(3)]
    hv_pad = [nc.alloc_sbuf_tensor(f"hv_pad{i}", [MC, MACRO * FB], BF).ap()
              for i in range(3)]
    koT = [nc.alloc_sbuf_tensor(f"koT{i}", [ND + 1, MACRO * MC], BF).ap()
           for i in range(3)]


    def blocks(ap, total, p0, npos, off, width=F):
        v = ap[:, 0:total * FB].rearrange("p (t f) -> p t f", f=FB)
        return v[:, p0:p0 + npos, off:off + width]

    def bcast_cols(ap_col, n):
        """[P, B, 1] view -> [P, B, n] stride-0 broadcast."""
        return AP(ap_col.tensor, ap_col.offset,
                  [ap_col.ap[0], ap_col.ap[1], [0, n]])

    with tile.TileContext(nc) as tc:
        with (
            tc.tile_pool(name="const", bufs=1) as cpool,
            tc.tile_pool(name="io", bufs=6) as iopool,
            tc.tile_pool(name="work", bufs=5) as wpool,
            tc.tile_pool(name="zi", bufs=4) as zpool,
            tc.tile_pool(name="pw", bufs=3, space="PSUM") as pwpool,
            tc.tile_pool(name="ps", bufs=1, space="PSUM") as spool,
            tc.tile_pool(name="phu", bufs=2, space="PSUM") as hupool,
        ):
            # ---- constants to SBUF ----
            wv = cpool.tile([128, 3 * MC], BF, tag="wv")
            wk = cpool.tile([128, 3 * MC], BF, tag="wk")
            wkq = cpool.tile([MC, 3 * MC], BF, tag="wkq")
            wo = cpool.tile([MC, 3 * C], BF, tag="wo")
            hmt = cpool.tile([ND + 1, ND], BF, tag="hmt")
            hma = cpool.tile([ND, ND + 1], BF, tag="hma")
            logm = cpool.tile([ND, 1], F32, tag="logm")
            nav = cpool.tile([128, 1], F32, tag="nav")
            nbv = cpool.tile([128, 1], F32, tag="nbv")
            t2s = cpool.tile([128, 1], F32, tag="t2s")
            t2b = cpool.tile([128, 1], F32, tag="t2b")
            bvv = cpool.tile([MC, 1], F32, tag="bvv")
            bkqv = cpool.tile([MC, 1], F32, tag="bkqv")
            bov = cpool.tile([C, 1], F32, tag="bov")
            for dst, src in ((wv, wv_d), (wk, wk_d), (wkq, wkq_d),
                             (wo, wo_d), (hmt, hmt_d), (hma, hma_d),
                             (logm, logm_d), (nav, nav_d), (nbv, nbv_d),
                             (t2s, t2s_d), (t2b, t2b_d), (bvv, bv_d),
                             (bkqv, bkq_d), (bov, bo_d)):
                nc.sync.dma_start(dst[:], src[:])

            # ---- init persistent tiles ----
            for i in range(3):
                nc.gpsimd.memset(t_pad[i][:], 0.0)
                nc.gpsimd.memset(kx_pad[i][:], 0.0)
                nc.gpsimd.memset(hq_pad[i][:], 0.0)
                nc.gpsimd.memset(hv_pad[i][:], 0.0)

                nc.sync.dma_start(koT[i][ND:ND + 1, :], bkrow_d[:])

            # ---- macro loop: software-pipelined over 7 stages ----
            # stage A(m): x DMA        [SP]
            # stage B(m): tanh/t2/kx   [ACT/Pool]
            # stage C(m): conv_v+move, conv_k+cast [PE/Pool/DVE]
            # stage D(m): scores+exp   [PE/ACT]
            # stage E(m): hproj+norm   [PE/DVE]
            # stage F(m): conv_kq+hv   [PE/Pool]
            # stage G(m): conv_o+out+y [PE/Pool/SP]
            RING = 3
            x_ring = {}
            xr_ring = {}
            v_ring = {}
            E_ring = {}

            def stage_a(m):
                x_t = iopool.tile([128, SUB * F], F32, tag="x")
                x_ring[m] = (x_t, 0)
                nc.sync.dma_start(x_t[:], x_d[m * 128:(m + 1) * 128, :])
                x_u = iopool.tile([C, MACRO * F], F32, tag="xu")
                xr_ring[m] = (x_u, 0)
                nc.sync.dma_start(x_u[:], x2_d[m * C:(m + 1) * C, :])

            def stage_b(m):
                tp = t_pad[m % RING]
                kp = kx_pad[m % RING]
                x_t, half = x_ring.pop(m)
                x3 = x_t[:].rearrange("p (t f) -> p t f", f=F)
                nc.scalar.activation(blocks(tp, SUB, 0, SUB, 1), x3,
                                     AF.Tanh, bias=nbv[:], scale=nav[:])
                t2 = wpool.tile([128, SUB * F], BF, tag="t2")
                t23 = t2.rearrange("p (t f) -> p t f", f=F)
                nc.vector.tensor_tensor(t23, blocks(tp, SUB, 0, SUB, 1),
                                        blocks(tp, SUB, 0, SUB, 1), ALU.mult)
                nc.scalar.activation(blocks(kp, SUB, 0, SUB, 1), t23,
                                     AF.Tanh, bias=t2b[:], scale=t2s[:])

            def stage_c(m):
                tp = t_pad[m % RING]
                kp = kx_pad[m % RING]
                koTb = koT[m % RING]
                v_sb = wpool.tile([MC, MACRO * F], BF, tag="v")
                v_ring[m] = v_sb
                for s in range(2):
                    pb = 64 * s
                    cs = s * SUB * F
                    v_ps = pwpool.tile([MC, 512], F32, tag="pw")
                    for d in range(3):
                        nc.tensor.matmul(
                            v_ps[:, 0:SUB * F],
                            wv[pb:pb + 64, d * MC:(d + 1) * MC],
                            blocks(tp, SUB, 0, SUB, d)[pb:pb + 64],
                            start=(d == 0), stop=(d == 2))
                    nc.scalar.activation(
                        v_sb[:, cs:cs + SUB * F], v_ps[:, 0:SUB * F],
                        AF.Identity, bias=bvv[:])
                    k_ps = pwpool.tile([MC, 512], F32, tag="pw")
                    for pr in range(SUB):
                        for d in range(3):
                            nc.tensor.matmul(
                                k_ps[0:F, pr * MC:(pr + 1) * MC],
                                blocks(kp, SUB, pr, 1, d)[pb:pb + 64],
                                wk[pb:pb + 64, d * MC:(d + 1) * MC],
                                start=(d == 0), stop=(d == 2))
                    nc.vector.tensor_copy(
                        koTb[0:F, s * 512:(s + 1) * 512], k_ps[0:F, 0:512])

            def stage_d(m):
                koTb = koT[m % RING]
                E = wpool.tile([ND, MACRO * MC], BF, tag="E")
                E_ring[m] = E
                s_ps = spool.tile([ND, MACRO * MC], F32, tag="s")
                for s in range(2):
                    nc.tensor.matmul(
                        s_ps[:, s * 512:(s + 1) * 512], hmt[:],
                        koTb[:, s * 512:(s + 1) * 512],
                        start=True, stop=True)
                nc.scalar.activation(E[:], s_ps[:], AF.Exp, bias=logm[:])

            def stage_e(m):
                E = E_ring.pop(m)
                hqp = hq_pad[m % RING]
                for s in range(2):
                    hu_ps = hupool.tile([MC, SUB * (F + 1)], F32, tag="hu")
                    for pr in range(SUB):
                        p = s * SUB + pr
                        nc.tensor.matmul(
                            hu_ps[:, pr * (F + 1):(pr + 1) * (F + 1)],
                            E[:, p * MC:(p + 1) * MC], hma[:],
                            start=True, stop=True)
                    hu3 = hu_ps[:, 0:SUB * (F + 1)].rearrange(
                        "p (t f) -> p t f", f=F + 1)
                    zi = zpool.tile([MC, SUB], F32, tag="zi")
                    zi3 = zi.rearrange("p (t f) -> p t f", f=1)
                    nc.vector.reciprocal(zi3, hu3[:, :, F:F + 1])
                    nc.vector.tensor_tensor(
                        blocks(hqp, MACRO, s * SUB, SUB, 1), hu3[:, :, 0:F],
                        bcast_cols(zi3, F), ALU.mult)

            def stage_f(m):
                hqp = hq_pad[m % RING]
                hvp = hv_pad[m % RING]
                v_sb = v_ring.pop(m)
                for s in range(2):
                    cs = s * SUB * F
                    h2_ps = pwpool.tile([MC, 512], F32, tag="pw")
                    for d in range(3):
                        nc.tensor.matmul(
                            h2_ps[:, 0:SUB * F],
                            wkq[:, d * MC:(d + 1) * MC],
                            blocks(hqp, MACRO, s * SUB, SUB, d),
                            start=(d == 0), stop=(d == 2))
                    h2s = wpool.tile([MC, SUB * F], BF, tag="h2s")
                    nc.scalar.activation(h2s[:], h2_ps[:, 0:SUB * F],
                                         AF.Identity, bias=bkqv[:])
                    nc.vector.tensor_tensor(
                        blocks(hvp, MACRO, s * SUB, SUB, 1),
                        h2s[:].rearrange("p (t f) -> p t f", f=F),
                        v_sb[:, cs:cs + SUB * F].rearrange(
                            "p (t f) -> p t f", f=F),
                        ALU.mult)

            def stage_g(m):
                hvp = hv_pad[m % RING]
                x_u, uh = xr_ring.pop(m)
                ub = 0
                out_sb = iopool.tile([C, MACRO * F], F32, tag="out")
                for s in range(2):
                    cs = s * SUB * F
                    o_ps = pwpool.tile([MC, 512], F32, tag="pw")
                    for d in range(3):
                        nc.tensor.matmul(
                            o_ps[0:C, 0:SUB * F],
                            wo[:, d * C:(d + 1) * C],
                            blocks(hvp, MACRO, s * SUB, SUB, d),
                            start=(d == 0), stop=(d == 2))
                    nc.vector.scalar_tensor_tensor(
                        out_sb[:, cs:cs + SUB * F], o_ps[0:C, 0:SUB * F],
                        bov[:], x_u[:, ub + cs:ub + cs + SUB * F],
                        ALU.add, ALU.add)
                nc.sync.dma_start(y_d[m * C:(m + 1) * C, :], out_sb[:])

            stages = [stage_a, stage_b, stage_c, stage_d, stage_e,
                      stage_f, stage_g]
            nstg = len(stages)
            loop_cm = tc.For_i(0, repeat, 1) if repeat > 1 else contextlib.nullcontext()
            with loop_cm:
                for i in range(nmacro + nstg - 1):
                    for si in range(nstg):
                        m = i - si
                        if 0 <= m < nmacro:
                            stages[si](m)

    nc.compile()
    return nc


def _prep_consts(inputs):
    f32 = np.float32
    na = f32(np.asarray(inputs["na"]).ravel()[0])
    na1 = f32(np.asarray(inputs["na1"]).ravel()[0])
    nb = np.asarray(inputs["nb"], f32).reshape(C)
    nb1 = np.asarray(inputs["nb1"], f32).reshape(C)
    ma = f32(np.asarray(inputs["ma"]).ravel()[0])
    ma1 = f32(np.asarray(inputs["ma1"]).ravel()[0])
    mb = np.asarray(inputs["mb"], f32).reshape(C)
    mb1 = np.asarray(inputs["mb1"], f32).reshape(C)
    Wv = np.asarray(inputs["Wv"], f32)
    bv = np.asarray(inputs["bv"], f32)
    Wk = np.asarray(inputs["Wk"], f32)
    bk = np.asarray(inputs["bk"], f32)
    Wkq = np.asarray(inputs["Wkq"], f32)
    bkq = np.asarray(inputs["bkq"], f32)
    Wo = np.asarray(inputs["Wo"], f32)
    bo = np.asarray(inputs["bo"], f32)
    Q = np.asarray(inputs["h_mat"], f32)

    assert np.all(nb1 == 0.0), "general nb1 path not implemented"
    assert np.all(mb1 == 0.0), "general mb1 path not implemented"

    Qd, cnt = np.unique(Q, axis=0, return_counts=True)
    assert Qd.shape[0] == ND, f"distinct rows {Qd.shape[0]} != {ND}"
    logm = np.log(cnt.astype(f32)).reshape(ND, 1).astype(f32)
    S = Qd.sum(axis=1)

    def dup(vec):
        return np.concatenate([vec, vec]).reshape(128, 1).astype(f32)

    wv = np.zeros((128, 3, MC), BF16)
    wk = np.zeros((128, 3, MC), BF16)
    wkq = np.zeros((MC, 3, MC), BF16)
    wo = np.zeros((MC, 3, C), BF16)
    for d in range(3):
        wv[0:64, d] = (na1 * Wv[:, :, 0, d]).T.astype(BF16)
        wv[64:128, d] = wv[0:64, d]
        wk[0:64, d] = (ma1 * Wk[:, :, 0, d]).T.astype(BF16)
        wk[64:128, d] = wk[0:64, d]
        wkq[:, d] = Wkq[:, :, 0, d].T.astype(BF16)
        wo[:, d] = Wo[:, :, 0, d].T.astype(BF16)

    hmt = np.zeros((ND + 1, ND), BF16)
    hmt[0:ND, :] = Qd.T.astype(BF16)
    hmt[ND, :] = S.astype(BF16)
    hma = np.zeros((ND, ND + 1), BF16)
    hma[:, 0:ND] = Qd.astype(BF16)
    hma[:, ND] = 1.0

    bkrow = np.tile(bk.astype(BF16), MACRO).reshape(1, MACRO * MC)

    return {
        "wv": wv.reshape(128, 3 * MC), "wk": wk.reshape(128, 3 * MC),
        "wkq": wkq.reshape(MC, 3 * MC), "wo": wo.reshape(MC, 3 * C),
        "hmt": hmt, "hma": hma, "logm": logm, "bkrow": bkrow,
        "nav": np.full((128, 1), na, f32), "nbv": dup(nb),
        "t2s": np.full((128, 1), ma * na1 * na1, f32), "t2b": dup(mb),
        "bvv": bv.reshape(MC, 1).astype(f32),
        "bkqv": bkq.reshape(MC, 1).astype(f32),
        "bov": bo.reshape(C, 1).astype(f32),
    }


def run(inputs, trace=False, repeat=1):
    x = np.asarray(inputs["x"], np.float32)
    B, _, T, _ = x.shape
    n_cores = 8
    splits = n_cores // B
    t_core = T // splits

    consts = _prep_consts(inputs)
    key = (t_core, repeat)
    if key not in _cache:
        _cache[key] = _build(t_core, repeat=repeat)
    nc = _cache[key]

    nmacro = t_core // MACRO
    in_maps = []
    for i in range(n_cores):
        b, t0 = i // splits, (i % splits) * t_core
        shard = x[b, :, t0:t0 + t_core, :].reshape(C, t_core * F)
        xt = np.ascontiguousarray(
            shard.reshape(C, nmacro, 2, SUB * F).transpose(1, 2, 0, 3)
        ).reshape(nmacro * 128, SUB * F)
        xu = np.ascontiguousarray(
            shard.reshape(C, nmacro, MACRO * F).transpose(1, 0, 2)
        ).reshape(nmacro * C, MACRO * F)
        in_maps.append({"x": xt, "x2": xu, **consts})

    res = run_bass_kernel_spmd(nc, in_maps, list(range(n_cores)), trace=trace)
    out = np.empty_like(x)
    for i in range(n_cores):
        b, t0 = i // splits, (i % splits) * t_core
        yt = res.results[i]["y"].reshape(nmacro, C, MACRO * F)
        out[b, :, t0:t0 + t_core, :] = np.ascontiguousarray(
            yt.transpose(1, 0, 2)).reshape(C, t_core, F)
    return out, res


def kernel(**inputs):
    out, _ = run(inputs)
    return out
